# revision 25
# baseline (speedup 1.0000x reference)
"""Trainium2 Bass kernel for AdaptiveInterpolationModule (dual-source cross-attention).

Reference computation (B=16, S=1024, D=768):
    Q   = x_C @ W_q.T + b_q
    K_s = x_s @ W_k.T + b_k          (s in {A, B})
    V_s = x_s @ W_v.T + b_v
    attn_s   = softmax(Q K_s^T / sqrt(D))
    interp_s = attn_s V_s
    h   = LayerNorm(interp_A + interp_B + x_C) * gamma + beta
    out = h @ W_fc.T + b_fc

Sharding: data-parallel over batch, 2 batches per core on 8 cores. No collectives.

Math simplifications (exact):
  - b_k never affects the output: scores rows shift by a k-constant -> softmax invariant.
  - b_v contributes exactly +b_v per source (attn rows sum to 1) -> folded into the
    residual input on the host (x_C + 2*b_v).
  - b_q is pre-scaled by 1/sqrt(D) on the host and added when casting Q to bf16.

Device dataflow per (core, batch):
  - x^T tiles (pre-transposed on host, bf16) + W^T tiles (pre-transposed, bf16).
  - Q^T, K_A^T, K_B^T computed transposed [e, s]; V_A, V_B computed natural [s, e]
    with two extra all-ones columns so the interp matmul also produces softmax
    row-sums for free.
  - scores^T[k, q] per q-block of 512: k on partitions; exp with no max subtraction
    (scores ~ N(0,1), overflow impossible).
  - interp accumulated over both sources' k-chunks; 1/rowsum applied as per-partition
    scalars during PSUM->SBUF copyout with the residual fused in; LayerNorm
    (bn_stats/bn_aggr); PE-transpose h with gamma/beta fused into the copyout;
    fc matmul; +b_fc; out.
  - Per q-block the epilogues are emitted in two passes (all interp+LN, then all
    transpose+fc) so the PE's static order never waits on a DVE chain.
"""

import sys

import numpy as np

try:
    import concourse.bass as bass
except ImportError:
    sys.path.insert(0, "/opt/trn_rl_repo")
    import concourse.bass as bass

import ml_dtypes
from contextlib import ExitStack

import concourse.mybir as mybir
import concourse.tile as tile
from concourse import bacc
from concourse.bass_utils import run_bass_kernel_spmd
from concourse.masks import make_identity

P = 128
DIM = 768
S = 1024
B = 16
NCORES = 8
BPC = B // NCORES  # batches per core
DCH = DIM // P     # 6 chunks of 128 along D
SCH = S // P       # 8 chunks of 128 along S
EPS = 1e-5
SCALE = 1.0 / float(np.sqrt(DIM))
F32 = mybir.dt.float32
BF16 = mybir.dt.bfloat16

AF = mybir.ActivationFunctionType
ALU = mybir.AluOpType

VW = DIM + 2  # V tile width: 768 value cols + 2 ones cols (row-sum trick)


def build_bass() -> bass.Bass:
    nc = bacc.Bacc()

    xaT = nc.declare_dram_parameter("xaT", [BPC, DIM, S], BF16, isOutput=False)
    xbT = nc.declare_dram_parameter("xbT", [BPC, DIM, S], BF16, isOutput=False)
    xcT = nc.declare_dram_parameter("xcT", [BPC, DIM, S], BF16, isOutput=False)
    xcr = nc.declare_dram_parameter("xcr", [BPC, S, DIM], F32, isOutput=False)
    wqT = nc.declare_dram_parameter("wqT", [DIM, DIM], BF16, isOutput=False)
    wkT = nc.declare_dram_parameter("wkT", [DIM, DIM], BF16, isOutput=False)
    wvT = nc.declare_dram_parameter("wvT", [DIM, DIM], BF16, isOutput=False)
    wfT = nc.declare_dram_parameter("wfT", [DIM, DIM], BF16, isOutput=False)
    bqs = nc.declare_dram_parameter("bqs", [DIM], F32, isOutput=False)
    bfc = nc.declare_dram_parameter("bfc", [DIM], F32, isOutput=False)
    out = nc.declare_dram_parameter("out", [BPC, S, DIM], F32, isOutput=True)

    with tile.TileContext(nc) as tc, ExitStack() as ctx:
        consts = ctx.enter_context(tc.tile_pool(name="consts", bufs=1))
        wpool = ctx.enter_context(tc.tile_pool(name="wpool", bufs=1))
        xpool = ctx.enter_context(tc.tile_pool(name="xpool", bufs=1))
        qkv = ctx.enter_context(tc.tile_pool(name="qkv", bufs=1))
        epool = ctx.enter_context(tc.tile_pool(name="epool", bufs=1))
        spool = ctx.enter_context(tc.tile_pool(name="spool", bufs=4))
        zpool = ctx.enter_context(tc.tile_pool(name="zpool", bufs=5))
        opool = ctx.enter_context(tc.tile_pool(name="opool", bufs=3))
        ps512 = ctx.enter_context(tc.tile_pool(name="ps512", bufs=4, space="PSUM"))
        ps258 = ctx.enter_context(tc.tile_pool(name="ps258", bufs=2, space="PSUM"))
        pstr = ctx.enter_context(tc.tile_pool(name="pstr", bufs=2, space="PSUM"))

        # --- first-needed DMAs first: Q projection needs xcT + wqT.
        # dma_start issue costs ~600ns of the issuing engine's sequencer, so
        # spread the initial loads across four otherwise-idle sequencers.
        dma_lanes = [nc.sync, nc.gpsimd]

        def load_xT(h, b, tag, lane0=0):
            t = xpool.tile([P, DCH, S], BF16, tag=tag)
            v = h[b].rearrange("(o p) s -> p o s", p=P)
            for do in range(DCH):
                eng = dma_lanes[(lane0 + do) % len(dma_lanes)]
                eng.dma_start(out=t[:, do, :], in_=v[:, do, :])
            return t

        def load_wT(h, tag, lane0=0):
            t = wpool.tile([P, DCH, DIM], BF16, tag=tag)
            v = h[:].rearrange("(o p) e -> p o e", p=P)
            for do in range(DCH):
                eng = dma_lanes[(lane0 + do) % len(dma_lanes)]
                eng.dma_start(out=t[:, do, :], in_=v[:, do, :])
            return t

        xc_b0 = load_xT(xcT, 0, "xcT", lane0=0)
        w_sb = {"q": load_wT(wqT, "wq", lane0=2)}

        bqs_sb = consts.tile([P, DCH], F32)
        nc.sync.dma_start(out=bqs_sb, in_=bqs[:].rearrange("(o p) -> p o", p=P))

        xa_b0 = load_xT(xaT, 0, "xaT", lane0=1)
        w_sb["k"] = load_wT(wkT, "wk", lane0=3)
        xb_b0 = load_xT(xbT, 0, "xbT", lane0=0)
        w_sb["v"] = load_wT(wvT, "wv", lane0=2)
        w_sb["f"] = load_wT(wfT, "wf", lane0=1)

        bfc_sb = consts.tile([P, DIM], F32)
        nc.sync.dma_start(out=bfc_sb, in_=bfc[:].partition_broadcast(P))

        idn = consts.tile([P, P], BF16)
        make_identity(nc, idn)
        eps_sb = consts.tile([P, 1], F32)
        nc.vector.memset(eps_sb, EPS)

        # PE warm-up: dummy transposes while input DMAs land. Keeps the HAM
        # clock-gate busy so real matmuls start at full clock, at no cost to
        # the critical path (no data dependencies, reuses the pstr psum tag).
        for _ in range(24):
            pst = pstr.tile([P, 2, P], BF16, tag="pstr")
            nc.tensor.transpose(pst[:, 0], idn, idn)
            nc.tensor.transpose(pst[:, 1], idn, idn)

        for b in range(BPC):
            if b == 0:
                x_sb = {"a": xa_b0, "b": xb_b0, "c": xc_b0}
            else:
                x_sb = {
                    "c": load_xT(xcT, b, "xcT", lane0=0),
                    "a": load_xT(xaT, b, "xaT", lane0=2),
                    "b": load_xT(xbT, b, "xbT", lane0=1),
                }

            # --- projections Q^T, K_A^T, K_B^T: [e, s] (e on partitions) ---
            def projT(tag, w_t, x_t, bias_ap=None):
                dst = qkv.tile([P, DCH, S], BF16, tag=tag)
                for ec in range(DCH):
                    for sh in range(S // 512):
                        ps = ps512.tile([P, 512], F32, tag="ps512")
                        for do in range(DCH):
                            nc.tensor.matmul(
                                ps,
                                lhsT=w_t[:, do, ec * P:(ec + 1) * P],
                                rhs=x_t[:, do, sh * 512:(sh + 1) * 512],
                                start=(do == 0),
                                stop=(do == DCH - 1),
                            )
                        o = dst[:, ec, sh * 512:(sh + 1) * 512]
                        if bias_ap is not None:
                            # out = psum * SCALE + (b_q * SCALE)
                            nc.scalar.activation(
                                out=o, in_=ps, func=AF.Identity,
                                bias=bias_ap[:, ec:ec + 1], scale=SCALE,
                            )
                        else:
                            nc.scalar.copy(out=o, in_=ps)
                return dst

            qT_sb = projT("QT", w_sb["q"], x_sb["c"], bias_ap=bqs_sb)
            kT = {
                "a": projT("KAT", w_sb["k"], x_sb["a"]),
                "b": projT("KBT", w_sb["k"], x_sb["b"]),
            }

            # --- V_A, V_B natural layout [s, e] + two ones columns ---
            v_sb = {}
            for name in ("a", "b"):
                dst = qkv.tile([P, SCH, VW], BF16, tag=f"V{name.upper()}")
                nc.vector.memset(dst[:, :, DIM:VW], 1.0)
                for sc in range(SCH):
                    for off, w in ((0, 512), (512, 256)):
                        pool, pw = (ps512, 512) if w == 512 else (ps258, 258)
                        ps = pool.tile([P, pw], F32, tag=f"ps{pw}")
                        for do in range(DCH):
                            nc.tensor.matmul(
                                ps[:, :w],
                                lhsT=x_sb[name][:, do, sc * P:(sc + 1) * P],
                                rhs=w_sb["v"][:, do, off:off + w],
                                start=(do == 0),
                                stop=(do == DCH - 1),
                            )
                        nc.scalar.copy(out=dst[:, sc, off:off + w], in_=ps[:, :w])
                v_sb[name] = dst

            # --- attention + epilogue, per q-block of 512 ---
            for qb in range(S // 512):
                qsl = slice(qb * 512, (qb + 1) * 512)
                # scores^T and exp: e^T[k, q] = exp(K[k,:] @ Qs[q,:])
                e_sb = {}
                for name in ("a", "b"):
                    et = epool.tile([P, SCH, 512], BF16, tag=f"e{name.upper()}")
                    for kc in range(SCH):
                        ps = ps512.tile([P, 512], F32, tag="ps512")
                        for eo in range(DCH):
                            nc.tensor.matmul(
                                ps,
                                lhsT=kT[name][:, eo, kc * P:(kc + 1) * P],
                                rhs=qT_sb[:, eo, qsl],
                                start=(eo == 0),
                                stop=(eo == DCH - 1),
                            )
                        nc.scalar.activation(out=et[:, kc, :], in_=ps, func=AF.Exp)
                    e_sb[name] = et

                # pass 1: interp + layernorm -> z[qi]
                zs = []
                for qi in range(4):
                    qc = qb * 4 + qi
                    qs = slice(qi * P, (qi + 1) * P)

                    xc_t = opool.tile([P, DIM], F32, tag="xc")
                    nc.gpsimd.dma_start(out=xc_t, in_=xcr[b, qc * P:(qc + 1) * P, :])

                    # interp psums; h1 carries the ones columns -> row-sums
                    pa = {}
                    for name in ("a", "b"):
                        p0 = ps512.tile([P, 512], F32, tag="ps512")
                        p1 = ps258.tile([P, 258], F32, tag="ps258")
                        for kc in range(SCH):
                            nc.tensor.matmul(
                                p0,
                                lhsT=e_sb[name][:, kc, qs],
                                rhs=v_sb[name][:, kc, 0:512],
                                start=(kc == 0),
                                stop=(kc == SCH - 1),
                            )
                        for kc in range(SCH):
                            nc.tensor.matmul(
                                p1,
                                lhsT=e_sb[name][:, kc, qs],
                                rhs=v_sb[name][:, kc, 512:VW],
                                start=(kc == 0),
                                stop=(kc == SCH - 1),
                            )
                        pa[name] = (p0, p1)

                    rcp = {}
                    for name in ("a", "b"):
                        r = spool.tile([P, 1], F32, tag=f"r{name}")
                        nc.vector.reciprocal(r, pa[name][1][:, 256:257])
                        rcp[name] = r

                    # t1 = psA*rA + xc ; t1 += psB*rB   (residual fused)
                    t1 = spool.tile([P, DIM], F32, tag="t1")
                    for (off, w, pi) in ((0, 512, 0), (512, 256, 1)):
                        nc.vector.scalar_tensor_tensor(
                            out=t1[:, off:off + w],
                            in0=pa["a"][pi][:, 0:w] if pi == 0 else pa["a"][1][:, 0:256],
                            scalar=rcp["a"], in1=xc_t[:, off:off + w],
                            op0=ALU.mult, op1=ALU.add,
                        )
                        nc.vector.scalar_tensor_tensor(
                            out=t1[:, off:off + w],
                            in0=pa["b"][pi][:, 0:w] if pi == 0 else pa["b"][1][:, 0:256],
                            scalar=rcp["b"], in1=t1[:, off:off + w],
                            op0=ALU.mult, op1=ALU.add,
                        )

                    # layernorm
                    stats = spool.tile([P, 3, 6], F32, tag="st")
                    for g in range(3):
                        nc.vector.bn_stats(
                            out=stats[:, g, :], in_=t1[:, g * 256:(g + 1) * 256]
                        )
                    mv = spool.tile([P, 2], F32, tag="mv")
                    nc.vector.bn_aggr(out=mv, in_=stats)
                    std = spool.tile([P, 1], F32, tag="std")
                    nc.scalar.activation(
                        out=std, in_=mv[:, 1:2], func=AF.Sqrt, bias=eps_sb
                    )
                    rstd = spool.tile([P, 1], F32, tag="rstd")
                    nc.vector.reciprocal(rstd, std)
                    z = zpool.tile([P, DIM], BF16, tag="z")
                    nc.vector.tensor_scalar(
                        out=z, in0=t1,
                        scalar1=mv[:, 0:1], scalar2=rstd,
                        op0=ALU.subtract, op1=ALU.mult,
                    )
                    zs.append(z)

                # pass 2: transpose h + fc + store
                for qi in range(4):
                    qc = qb * 4 + qi
                    z = zs[qi]

                    hT = opool.tile([P, DCH, P], BF16, tag="hT")
                    for ep in range(DCH // 2):
                        pst = pstr.tile([P, 2, P], BF16, tag="pstr")
                        for j in range(2):
                            eo = ep * 2 + j
                            nc.tensor.transpose(
                                pst[:, j], z[:, eo * P:(eo + 1) * P], idn
                            )
                        nc.scalar.copy(out=hT[:, ep * 2:(ep + 1) * 2, :], in_=pst)

                    o_t = opool.tile([P, DIM], F32, tag="o")
                    for off, w in ((0, 512), (512, 256)):
                        pool, pw = (ps512, 512) if w == 512 else (ps258, 258)
                        ps = pool.tile([P, pw], F32, tag=f"ps{pw}")
                        for eo in range(DCH):
                            nc.tensor.matmul(
                                ps[:, :w],
                                lhsT=hT[:, eo, :],
                                rhs=w_sb["f"][:, eo, off:off + w],
                                start=(eo == 0),
                                stop=(eo == DCH - 1),
                            )
                        nc.vector.tensor_add(
                            o_t[:, off:off + w], ps[:, :w], bfc_sb[:, off:off + w]
                        )
                    nc.sync.dma_start(out=out[b, qc * P:(qc + 1) * P, :], in_=o_t)

    nc.compile()
    return nc


_CACHED_NC = None
_LAST_IN_MAPS = None


def kernel(**inputs) -> np.ndarray:
    global _CACHED_NC, _LAST_IN_MAPS
    bf16 = ml_dtypes.bfloat16
    f32 = np.float32

    xA = np.asarray(inputs["x_A"], dtype=f32)
    xB = np.asarray(inputs["x_B"], dtype=f32)
    xC = np.asarray(inputs["x_C"], dtype=f32)

    xaT = np.ascontiguousarray(xA.transpose(0, 2, 1)).astype(bf16)
    xbT = np.ascontiguousarray(xB.transpose(0, 2, 1)).astype(bf16)
    xcT = np.ascontiguousarray(xC.transpose(0, 2, 1)).astype(bf16)
    xcr = (xC + 2.0 * np.asarray(inputs["b_v"], dtype=f32)).astype(f32)

    wqT = np.ascontiguousarray(np.asarray(inputs["W_q"], dtype=f32).T).astype(bf16)
    wkT = np.ascontiguousarray(np.asarray(inputs["W_k"], dtype=f32).T).astype(bf16)
    wvT = np.ascontiguousarray(np.asarray(inputs["W_v"], dtype=f32).T).astype(bf16)

    # fold LayerNorm's gamma/beta into the fc layer (exact):
    #   h = z*gamma + beta;  out = h @ W_fc.T + b_fc
    #     = z @ (W_fc * gamma).T + (b_fc + W_fc @ beta)
    gam = np.asarray(inputs["gamma"], dtype=f32)
    bet = np.asarray(inputs["beta"], dtype=f32)
    W_fc = np.asarray(inputs["W_fc"], dtype=f32)
    wfT = np.ascontiguousarray(W_fc.T * gam[:, None]).astype(bf16)
    bfc = (np.asarray(inputs["b_fc"], dtype=f32) + W_fc @ bet).astype(f32)

    bqs = (np.asarray(inputs["b_q"], dtype=f32) * SCALE).astype(f32)

    if _CACHED_NC is None:
        _CACHED_NC = build_bass()
    nc = _CACHED_NC

    in_maps = []
    for c in range(NCORES):
        sl = slice(c * BPC, (c + 1) * BPC)
        in_maps.append({
            "xaT": np.ascontiguousarray(xaT[sl]),
            "xbT": np.ascontiguousarray(xbT[sl]),
            "xcT": np.ascontiguousarray(xcT[sl]),
            "xcr": np.ascontiguousarray(xcr[sl]),
            "wqT": wqT, "wkT": wkT, "wvT": wvT, "wfT": wfT,
            "bqs": bqs, "bfc": bfc,
        })

    _LAST_IN_MAPS = in_maps
    res = run_bass_kernel_spmd(nc, in_maps, core_ids=list(range(NCORES)))
    outs = [np.asarray(res.results[i]["out"], dtype=f32) for i in range(NCORES)]
    return np.concatenate(outs, axis=0)


if __name__ == "__main__":
    rng = np.random.default_rng(0)
    fake = {
        "x_A": rng.standard_normal((B, S, DIM), dtype=np.float32),
        "x_B": rng.standard_normal((B, S, DIM), dtype=np.float32),
        "x_C": rng.standard_normal((B, S, DIM), dtype=np.float32),
        "W_q": rng.standard_normal((DIM, DIM), dtype=np.float32) / 27.7,
        "b_q": rng.standard_normal(DIM).astype(np.float32) / 27.7,
        "W_k": rng.standard_normal((DIM, DIM), dtype=np.float32) / 27.7,
        "b_k": rng.standard_normal(DIM).astype(np.float32) / 27.7,
        "W_v": rng.standard_normal((DIM, DIM), dtype=np.float32) / 27.7,
        "b_v": rng.standard_normal(DIM).astype(np.float32) / 27.7,
        "gamma": np.ones(DIM, np.float32),
        "beta": np.zeros(DIM, np.float32),
        "W_fc": rng.standard_normal((DIM, DIM), dtype=np.float32) / 27.7,
        "b_fc": rng.standard_normal(DIM).astype(np.float32) / 27.7,
    }
    o = kernel(**fake)
    print(o.shape, o.dtype)


# revision 32
# speedup vs baseline: 1.0967x; 1.0967x over previous
"""Trainium2 Bass kernel for AdaptiveInterpolationModule (dual-source cross-attention).

Reference computation (B=16, S=1024, D=768):
    Q   = x_C @ W_q.T + b_q
    K_s = x_s @ W_k.T + b_k          (s in {A, B})
    V_s = x_s @ W_v.T + b_v
    attn_s   = softmax(Q K_s^T / sqrt(D))
    interp_s = attn_s V_s
    h   = LayerNorm(interp_A + interp_B + x_C) * gamma + beta
    out = h @ W_fc.T + b_fc

Sharding: data-parallel over batch, 2 batches per core on 8 cores. No collectives.

Math simplifications (exact):
  - b_k never affects the output: scores rows shift by a k-constant -> softmax invariant.
  - b_v contributes exactly +b_v per source (attn rows sum to 1) -> folded into the
    residual input on the host (x_C + 2*b_v).
  - b_q is pre-scaled by 1/sqrt(D) on the host and added when casting Q to bf16.

Device dataflow per (core, batch):
  - x^T tiles (pre-transposed on host, bf16) + W^T tiles (pre-transposed, bf16).
  - Q^T, K_A^T, K_B^T computed transposed [e, s]; V_A, V_B computed natural [s, e]
    with two extra all-ones columns so the interp matmul also produces softmax
    row-sums for free.
  - scores^T[k, q] per q-block of 512: k on partitions; exp with no max subtraction
    (scores ~ N(0,1), overflow impossible).
  - interp accumulated over both sources' k-chunks; 1/rowsum applied as per-partition
    scalars during PSUM->SBUF copyout with the residual fused in; LayerNorm
    (bn_stats/bn_aggr); PE-transpose h with gamma/beta fused into the copyout;
    fc matmul; +b_fc; out.
  - Per q-block the epilogues are emitted in two passes (all interp+LN, then all
    transpose+fc) so the PE's static order never waits on a DVE chain.
"""

import sys

import numpy as np

try:
    import concourse.bass as bass
except ImportError:
    sys.path.insert(0, "/opt/trn_rl_repo")
    import concourse.bass as bass

import ml_dtypes
from contextlib import ExitStack

import concourse.mybir as mybir
import concourse.tile as tile
from concourse import bacc
from concourse.bass_utils import run_bass_kernel_spmd
from concourse.masks import make_identity

P = 128
DIM = 768
S = 1024
B = 16
NCORES = 8
BPC = B // NCORES  # batches per core
DCH = DIM // P     # 6 chunks of 128 along D
SCH = S // P       # 8 chunks of 128 along S
EPS = 1e-5
SCALE = 1.0 / float(np.sqrt(DIM))
F32 = mybir.dt.float32
BF16 = mybir.dt.bfloat16
FP8 = mybir.dt.float8e4
# constant softmax shift: exp(score - ESHIFT) keeps values inside fp8e4m3
# range (max score on N(0,1)-scaled data is ~8 -> exp(4.5) = 90 < 448).
# Softmax is shift-invariant so this is exact.
ESHIFT = 3.5

AF = mybir.ActivationFunctionType
ALU = mybir.AluOpType

VW = DIM + 2  # V tile width: 768 value cols + 2 ones cols (row-sum trick)


def build_bass() -> bass.Bass:
    nc = bacc.Bacc()

    xaT = nc.declare_dram_parameter("xaT", [BPC, DIM, S], BF16, isOutput=False)
    xbT = nc.declare_dram_parameter("xbT", [BPC, DIM, S], BF16, isOutput=False)
    xcT = nc.declare_dram_parameter("xcT", [BPC, DIM, S], BF16, isOutput=False)
    xcr = nc.declare_dram_parameter("xcr", [BPC, S, DIM], F32, isOutput=False)
    wqT = nc.declare_dram_parameter("wqT", [DIM, DIM], BF16, isOutput=False)
    wkT = nc.declare_dram_parameter("wkT", [DIM, DIM], BF16, isOutput=False)
    wvT = nc.declare_dram_parameter("wvT", [DIM, DIM], BF16, isOutput=False)
    wfT = nc.declare_dram_parameter("wfT", [DIM, DIM], BF16, isOutput=False)
    bqs = nc.declare_dram_parameter("bqs", [DIM], F32, isOutput=False)
    bfc = nc.declare_dram_parameter("bfc", [DIM], F32, isOutput=False)
    out = nc.declare_dram_parameter("out", [BPC, S, DIM], F32, isOutput=True)

    with tile.TileContext(nc) as tc, ExitStack() as ctx:
        consts = ctx.enter_context(tc.tile_pool(name="consts", bufs=1))
        wpool = ctx.enter_context(tc.tile_pool(name="wpool", bufs=1))
        xpool = ctx.enter_context(tc.tile_pool(name="xpool", bufs=1))
        qkv = ctx.enter_context(tc.tile_pool(name="qkv", bufs=1))
        epool = ctx.enter_context(tc.tile_pool(name="epool", bufs=1))
        spool = ctx.enter_context(tc.tile_pool(name="spool", bufs=4))
        zpool = ctx.enter_context(tc.tile_pool(name="zpool", bufs=5))
        opool = ctx.enter_context(tc.tile_pool(name="opool", bufs=3))
        ps512 = ctx.enter_context(tc.tile_pool(name="ps512", bufs=4, space="PSUM"))
        ps258 = ctx.enter_context(tc.tile_pool(name="ps258", bufs=2, space="PSUM"))
        pstr = ctx.enter_context(tc.tile_pool(name="pstr", bufs=2, space="PSUM"))

        # --- first-needed DMAs first: Q projection needs xcT + wqT.
        # dma_start issue costs ~600ns of the issuing engine's sequencer, so
        # spread the initial loads across four otherwise-idle sequencers.
        dma_lanes = [nc.sync, nc.gpsimd]

        def load_xT(h, b, tag, lane0=0):
            t = xpool.tile([P, DCH, S], BF16, tag=tag)
            v = h[b].rearrange("(o p) s -> p o s", p=P)
            for do in range(DCH):
                eng = dma_lanes[(lane0 + do) % len(dma_lanes)]
                eng.dma_start(out=t[:, do, :], in_=v[:, do, :])
            return t

        def load_wT(h, tag, lane0=0):
            t = wpool.tile([P, DCH, DIM], BF16, tag=tag)
            v = h[:].rearrange("(o p) e -> p o e", p=P)
            for do in range(DCH):
                eng = dma_lanes[(lane0 + do) % len(dma_lanes)]
                eng.dma_start(out=t[:, do, :], in_=v[:, do, :])
            return t

        xc_b0 = load_xT(xcT, 0, "xcT", lane0=0)
        w_sb = {"q": load_wT(wqT, "wq", lane0=2)}

        bqs_sb = consts.tile([P, DCH], F32)
        nc.sync.dma_start(out=bqs_sb, in_=bqs[:].rearrange("(o p) -> p o", p=P))

        xa_b0 = load_xT(xaT, 0, "xaT", lane0=1)
        w_sb["k"] = load_wT(wkT, "wk", lane0=3)
        xb_b0 = load_xT(xbT, 0, "xbT", lane0=0)
        w_sb["v"] = load_wT(wvT, "wv", lane0=2)
        w_sb["f"] = load_wT(wfT, "wf", lane0=1)

        bfc_sb = consts.tile([P, DIM], F32)
        nc.sync.dma_start(out=bfc_sb, in_=bfc[:].partition_broadcast(P))

        idn = consts.tile([P, P], BF16)
        make_identity(nc, idn)
        eps_sb = consts.tile([P, 1], F32)
        nc.vector.memset(eps_sb, EPS)
        eshift_sb = consts.tile([P, 1], F32)
        nc.vector.memset(eshift_sb, -ESHIFT)

        for b in range(BPC):
            if b == 0:
                x_sb = {"a": xa_b0, "b": xb_b0, "c": xc_b0}
            else:
                x_sb = {
                    "c": load_xT(xcT, b, "xcT", lane0=0),
                    "a": load_xT(xaT, b, "xaT", lane0=2),
                    "b": load_xT(xbT, b, "xbT", lane0=1),
                }

            # --- projections Q^T, K_A^T, K_B^T: [e, s] (e on partitions) ---
            def projT(tag, w_t, x_t, bias_ap=None):
                dst = qkv.tile([P, DCH, S], BF16, tag=tag)
                for ec in range(DCH):
                    for sh in range(S // 512):
                        ps = ps512.tile([P, 512], F32, tag="ps512")
                        for do in range(DCH):
                            nc.tensor.matmul(
                                ps,
                                lhsT=w_t[:, do, ec * P:(ec + 1) * P],
                                rhs=x_t[:, do, sh * 512:(sh + 1) * 512],
                                start=(do == 0),
                                stop=(do == DCH - 1),
                            )
                        o = dst[:, ec, sh * 512:(sh + 1) * 512]
                        if bias_ap is not None:
                            # out = psum * SCALE + (b_q * SCALE)
                            nc.scalar.activation(
                                out=o, in_=ps, func=AF.Identity,
                                bias=bias_ap[:, ec:ec + 1], scale=SCALE,
                            )
                        else:
                            nc.scalar.copy(out=o, in_=ps)
                return dst

            qT_sb = projT("QT", w_sb["q"], x_sb["c"], bias_ap=bqs_sb)
            kT = {
                "a": projT("KAT", w_sb["k"], x_sb["a"]),
                "b": projT("KBT", w_sb["k"], x_sb["b"]),
            }

            # --- V_A, V_B natural layout [s, e] + two ones columns ---
            v_sb = {}
            for name in ("a", "b"):
                dst = qkv.tile([P, SCH, VW], FP8, tag=f"V{name.upper()}")
                nc.vector.memset(dst[:, :, DIM:VW], 1.0)
                for sc in range(SCH):
                    for off, w in ((0, 512), (512, 256)):
                        pool, pw = (ps512, 512) if w == 512 else (ps258, 258)
                        ps = pool.tile([P, pw], F32, tag=f"ps{pw}")
                        for do in range(DCH):
                            nc.tensor.matmul(
                                ps[:, :w],
                                lhsT=x_sb[name][:, do, sc * P:(sc + 1) * P],
                                rhs=w_sb["v"][:, do, off:off + w],
                                start=(do == 0),
                                stop=(do == DCH - 1),
                            )
                        nc.scalar.copy(out=dst[:, sc, off:off + w], in_=ps[:, :w])
                v_sb[name] = dst

            # --- attention + epilogue, per q-block of 512 ---
            for qb in range(S // 512):
                qsl = slice(qb * 512, (qb + 1) * 512)
                # scores^T and exp: e^T[k, q] = exp(K[k,:] @ Qs[q,:])
                e_sb = {}
                for name in ("a", "b"):
                    et = epool.tile([P, SCH, 512], FP8, tag=f"e{name.upper()}")
                    for kc in range(SCH):
                        ps = ps512.tile([P, 512], F32, tag="ps512")
                        for eo in range(DCH):
                            nc.tensor.matmul(
                                ps,
                                lhsT=kT[name][:, eo, kc * P:(kc + 1) * P],
                                rhs=qT_sb[:, eo, qsl],
                                start=(eo == 0),
                                stop=(eo == DCH - 1),
                            )
                        nc.scalar.activation(
                            out=et[:, kc, :], in_=ps, func=AF.Exp, bias=eshift_sb
                        )
                    e_sb[name] = et

                # pass 1: interp + layernorm -> z[qi]
                zs = []
                for qi in range(4):
                    qc = qb * 4 + qi
                    qs = slice(qi * P, (qi + 1) * P)

                    xc_t = opool.tile([P, DIM], F32, tag="xc")
                    nc.gpsimd.dma_start(out=xc_t, in_=xcr[b, qc * P:(qc + 1) * P, :])

                    # interp psums; h1 carries the ones columns -> row-sums
                    pa = {}
                    for name in ("a", "b"):
                        p0 = ps512.tile([P, 512], F32, tag="ps512")
                        p1 = ps258.tile([P, 258], F32, tag="ps258")
                        for kp in range(SCH // 2):
                            ksl = slice(2 * kp, 2 * kp + 2)
                            nc.tensor.matmul(
                                p0,
                                lhsT=e_sb[name][:, ksl, qs],
                                rhs=v_sb[name][:, ksl, 0:512],
                                start=(kp == 0),
                                stop=(kp == SCH // 2 - 1),
                                perf_mode=mybir.MatmulPerfMode.DoubleRow,
                            )
                        for kp in range(SCH // 2):
                            ksl = slice(2 * kp, 2 * kp + 2)
                            nc.tensor.matmul(
                                p1,
                                lhsT=e_sb[name][:, ksl, qs],
                                rhs=v_sb[name][:, ksl, 512:VW],
                                start=(kp == 0),
                                stop=(kp == SCH // 2 - 1),
                                perf_mode=mybir.MatmulPerfMode.DoubleRow,
                            )
                        pa[name] = (p0, p1)

                    rcp = {}
                    for name in ("a", "b"):
                        r = spool.tile([P, 1], F32, tag=f"r{name}")
                        nc.vector.reciprocal(r, pa[name][1][:, 256:257])
                        rcp[name] = r

                    # t1 = psA*rA + xc ; t1 += psB*rB   (residual fused)
                    t1 = spool.tile([P, DIM], F32, tag="t1")
                    for (off, w, pi) in ((0, 512, 0), (512, 256, 1)):
                        nc.vector.scalar_tensor_tensor(
                            out=t1[:, off:off + w],
                            in0=pa["a"][pi][:, 0:w] if pi == 0 else pa["a"][1][:, 0:256],
                            scalar=rcp["a"], in1=xc_t[:, off:off + w],
                            op0=ALU.mult, op1=ALU.add,
                        )
                        nc.vector.scalar_tensor_tensor(
                            out=t1[:, off:off + w],
                            in0=pa["b"][pi][:, 0:w] if pi == 0 else pa["b"][1][:, 0:256],
                            scalar=rcp["b"], in1=t1[:, off:off + w],
                            op0=ALU.mult, op1=ALU.add,
                        )

                    # layernorm
                    stats = spool.tile([P, 3, 6], F32, tag="st")
                    for g in range(3):
                        nc.vector.bn_stats(
                            out=stats[:, g, :], in_=t1[:, g * 256:(g + 1) * 256]
                        )
                    mv = spool.tile([P, 2], F32, tag="mv")
                    nc.vector.bn_aggr(out=mv, in_=stats)
                    std = spool.tile([P, 1], F32, tag="std")
                    nc.scalar.activation(
                        out=std, in_=mv[:, 1:2], func=AF.Sqrt, bias=eps_sb
                    )
                    rstd = spool.tile([P, 1], F32, tag="rstd")
                    nc.vector.reciprocal(rstd, std)
                    z = zpool.tile([P, DIM], BF16, tag="z")
                    nc.vector.tensor_scalar(
                        out=z, in0=t1,
                        scalar1=mv[:, 0:1], scalar2=rstd,
                        op0=ALU.subtract, op1=ALU.mult,
                    )
                    zs.append(z)

                # pass 2: transpose h + fc + store
                for qi in range(4):
                    qc = qb * 4 + qi
                    z = zs[qi]

                    hT = opool.tile([P, DCH, P], BF16, tag="hT")
                    for ep in range(DCH // 2):
                        pst = pstr.tile([P, 2, P], BF16, tag="pstr")
                        for j in range(2):
                            eo = ep * 2 + j
                            nc.tensor.transpose(
                                pst[:, j], z[:, eo * P:(eo + 1) * P], idn
                            )
                        nc.scalar.copy(out=hT[:, ep * 2:(ep + 1) * 2, :], in_=pst)

                    o_t = opool.tile([P, DIM], F32, tag="o")
                    for off, w in ((0, 512), (512, 256)):
                        pool, pw = (ps512, 512) if w == 512 else (ps258, 258)
                        ps = pool.tile([P, pw], F32, tag=f"ps{pw}")
                        for eo in range(DCH):
                            nc.tensor.matmul(
                                ps[:, :w],
                                lhsT=hT[:, eo, :],
                                rhs=w_sb["f"][:, eo, off:off + w],
                                start=(eo == 0),
                                stop=(eo == DCH - 1),
                            )
                        nc.vector.tensor_add(
                            o_t[:, off:off + w], ps[:, :w], bfc_sb[:, off:off + w]
                        )
                    nc.sync.dma_start(out=out[b, qc * P:(qc + 1) * P, :], in_=o_t)

    nc.compile()
    return nc


_CACHED_NC = None
_LAST_IN_MAPS = None


def kernel(**inputs) -> np.ndarray:
    global _CACHED_NC, _LAST_IN_MAPS
    bf16 = ml_dtypes.bfloat16
    f32 = np.float32

    xA = np.asarray(inputs["x_A"], dtype=f32)
    xB = np.asarray(inputs["x_B"], dtype=f32)
    xC = np.asarray(inputs["x_C"], dtype=f32)

    xaT = np.ascontiguousarray(xA.transpose(0, 2, 1)).astype(bf16)
    xbT = np.ascontiguousarray(xB.transpose(0, 2, 1)).astype(bf16)
    xcT = np.ascontiguousarray(xC.transpose(0, 2, 1)).astype(bf16)
    xcr = (xC + 2.0 * np.asarray(inputs["b_v"], dtype=f32)).astype(f32)

    wqT = np.ascontiguousarray(np.asarray(inputs["W_q"], dtype=f32).T).astype(bf16)
    wkT = np.ascontiguousarray(np.asarray(inputs["W_k"], dtype=f32).T).astype(bf16)
    wvT = np.ascontiguousarray(np.asarray(inputs["W_v"], dtype=f32).T).astype(bf16)

    # fold LayerNorm's gamma/beta into the fc layer (exact):
    #   h = z*gamma + beta;  out = h @ W_fc.T + b_fc
    #     = z @ (W_fc * gamma).T + (b_fc + W_fc @ beta)
    gam = np.asarray(inputs["gamma"], dtype=f32)
    bet = np.asarray(inputs["beta"], dtype=f32)
    W_fc = np.asarray(inputs["W_fc"], dtype=f32)
    wfT = np.ascontiguousarray(W_fc.T * gam[:, None]).astype(bf16)
    bfc = (np.asarray(inputs["b_fc"], dtype=f32) + W_fc @ bet).astype(f32)

    bqs = (np.asarray(inputs["b_q"], dtype=f32) * SCALE).astype(f32)

    if _CACHED_NC is None:
        _CACHED_NC = build_bass()
    nc = _CACHED_NC

    in_maps = []
    for c in range(NCORES):
        sl = slice(c * BPC, (c + 1) * BPC)
        in_maps.append({
            "xaT": np.ascontiguousarray(xaT[sl]),
            "xbT": np.ascontiguousarray(xbT[sl]),
            "xcT": np.ascontiguousarray(xcT[sl]),
            "xcr": np.ascontiguousarray(xcr[sl]),
            "wqT": wqT, "wkT": wkT, "wvT": wvT, "wfT": wfT,
            "bqs": bqs, "bfc": bfc,
        })

    _LAST_IN_MAPS = in_maps
    res = run_bass_kernel_spmd(nc, in_maps, core_ids=list(range(NCORES)))
    outs = [np.asarray(res.results[i]["out"], dtype=f32) for i in range(NCORES)]
    return np.concatenate(outs, axis=0)


if __name__ == "__main__":
    rng = np.random.default_rng(0)
    fake = {
        "x_A": rng.standard_normal((B, S, DIM), dtype=np.float32),
        "x_B": rng.standard_normal((B, S, DIM), dtype=np.float32),
        "x_C": rng.standard_normal((B, S, DIM), dtype=np.float32),
        "W_q": rng.standard_normal((DIM, DIM), dtype=np.float32) / 27.7,
        "b_q": rng.standard_normal(DIM).astype(np.float32) / 27.7,
        "W_k": rng.standard_normal((DIM, DIM), dtype=np.float32) / 27.7,
        "b_k": rng.standard_normal(DIM).astype(np.float32) / 27.7,
        "W_v": rng.standard_normal((DIM, DIM), dtype=np.float32) / 27.7,
        "b_v": rng.standard_normal(DIM).astype(np.float32) / 27.7,
        "gamma": np.ones(DIM, np.float32),
        "beta": np.zeros(DIM, np.float32),
        "W_fc": rng.standard_normal((DIM, DIM), dtype=np.float32) / 27.7,
        "b_fc": rng.standard_normal(DIM).astype(np.float32) / 27.7,
    }
    o = kernel(**fake)
    print(o.shape, o.dtype)


# revision 35
# speedup vs baseline: 1.2395x; 1.1302x over previous
"""Trainium2 Bass kernel for AdaptiveInterpolationModule (dual-source cross-attention).

Reference computation (B=16, S=1024, D=768):
    Q   = x_C @ W_q.T + b_q
    K_s = x_s @ W_k.T + b_k          (s in {A, B})
    V_s = x_s @ W_v.T + b_v
    attn_s   = softmax(Q K_s^T / sqrt(D))
    interp_s = attn_s V_s
    h   = LayerNorm(interp_A + interp_B + x_C) * gamma + beta
    out = h @ W_fc.T + b_fc

Sharding: data-parallel over batch, 2 batches per core on 8 cores. No collectives.

Math simplifications (exact):
  - b_k never affects the output: scores rows shift by a k-constant -> softmax invariant.
  - b_v contributes exactly +b_v per source (attn rows sum to 1) -> folded into the
    residual input on the host (x_C + 2*b_v).
  - b_q is pre-scaled by 1/sqrt(D) on the host and added when casting Q to bf16.

Device dataflow per (core, batch):
  - x^T tiles (pre-transposed on host, bf16) + W^T tiles (pre-transposed, bf16).
  - Q^T, K_A^T, K_B^T computed transposed [e, s]; V_A, V_B computed natural [s, e]
    with two extra all-ones columns so the interp matmul also produces softmax
    row-sums for free.
  - scores^T[k, q] per q-block of 512: k on partitions; exp with no max subtraction
    (scores ~ N(0,1), overflow impossible).
  - interp accumulated over both sources' k-chunks; 1/rowsum applied as per-partition
    scalars during PSUM->SBUF copyout with the residual fused in; LayerNorm
    (bn_stats/bn_aggr); PE-transpose h with gamma/beta fused into the copyout;
    fc matmul; +b_fc; out.
  - Per q-block the epilogues are emitted in two passes (all interp+LN, then all
    transpose+fc) so the PE's static order never waits on a DVE chain.
"""

import sys

import numpy as np

try:
    import concourse.bass as bass
except ImportError:
    sys.path.insert(0, "/opt/trn_rl_repo")
    import concourse.bass as bass

import ml_dtypes
from contextlib import ExitStack

import concourse.mybir as mybir
import concourse.tile as tile
from concourse import bacc
from concourse.bass_utils import run_bass_kernel_spmd
from concourse.masks import make_identity

P = 128
DIM = 768
S = 1024
B = 16
NCORES = 8
BPC = B // NCORES  # batches per core
DCH = DIM // P     # 6 chunks of 128 along D
SCH = S // P       # 8 chunks of 128 along S
EPS = 1e-5
SCALE = 1.0 / float(np.sqrt(DIM))
F32 = mybir.dt.float32
BF16 = mybir.dt.bfloat16
FP8 = mybir.dt.float8e4
# constant softmax shift: exp(score - ESHIFT) keeps values inside fp8e4m3
# range (max score on N(0,1)-scaled data is ~8 -> exp(4.5) = 90 < 448).
# Softmax is shift-invariant so this is exact.
ESHIFT = 3.5

AF = mybir.ActivationFunctionType
ALU = mybir.AluOpType

VW = DIM + 2  # V tile width: 768 value cols + 2 ones cols (row-sum trick)


def build_bass() -> bass.Bass:
    nc = bacc.Bacc()

    xaT = nc.declare_dram_parameter("xaT", [BPC, DIM, S], BF16, isOutput=False)
    xbT = nc.declare_dram_parameter("xbT", [BPC, DIM, S], BF16, isOutput=False)
    xcT = nc.declare_dram_parameter("xcT", [BPC, DIM, S], BF16, isOutput=False)
    xcr = nc.declare_dram_parameter("xcr", [BPC, S, DIM], F32, isOutput=False)
    wqT = nc.declare_dram_parameter("wqT", [DIM, DIM], BF16, isOutput=False)
    wkT = nc.declare_dram_parameter("wkT", [DIM, DIM], BF16, isOutput=False)
    wvT = nc.declare_dram_parameter("wvT", [DIM, DIM], BF16, isOutput=False)
    wfT = nc.declare_dram_parameter("wfT", [DIM, DIM], BF16, isOutput=False)
    bqs = nc.declare_dram_parameter("bqs", [DIM], F32, isOutput=False)
    bfc = nc.declare_dram_parameter("bfc", [DIM], F32, isOutput=False)
    out = nc.declare_dram_parameter("out", [BPC, S, DIM], F32, isOutput=True)

    with tile.TileContext(nc) as tc, ExitStack() as ctx:
        consts = ctx.enter_context(tc.tile_pool(name="consts", bufs=1))
        wpool = ctx.enter_context(tc.tile_pool(name="wpool", bufs=1))
        xpool = ctx.enter_context(tc.tile_pool(name="xpool", bufs=1))
        qkv = ctx.enter_context(tc.tile_pool(name="qkv", bufs=1))
        epool = ctx.enter_context(tc.tile_pool(name="epool", bufs=1))
        spool = ctx.enter_context(tc.tile_pool(name="spool", bufs=4))
        zpool = ctx.enter_context(tc.tile_pool(name="zpool", bufs=5))
        opool = ctx.enter_context(tc.tile_pool(name="opool", bufs=3))
        ps512 = ctx.enter_context(tc.tile_pool(name="ps512", bufs=4, space="PSUM"))
        ps258 = ctx.enter_context(tc.tile_pool(name="ps258", bufs=2, space="PSUM"))
        pstr = ctx.enter_context(tc.tile_pool(name="pstr", bufs=2, space="PSUM"))

        # --- first-needed DMAs first: Q projection needs xcT + wqT.
        # dma_start issue costs ~600ns of the issuing engine's sequencer, so
        # spread the initial loads across four otherwise-idle sequencers.
        dma_lanes = [nc.sync, nc.gpsimd]

        def load_xT(h, b, tag, lane0=0):
            t = xpool.tile([P, DCH, S], BF16, tag=tag)
            v = h[b].rearrange("(o p) s -> p o s", p=P)
            for do in range(DCH):
                eng = dma_lanes[(lane0 + do) % len(dma_lanes)]
                eng.dma_start(out=t[:, do, :], in_=v[:, do, :])
            return t

        def load_wT(h, tag, lane0=0):
            t = wpool.tile([P, DCH, DIM], BF16, tag=tag)
            v = h[:].rearrange("(o p) e -> p o e", p=P)
            for do in range(DCH):
                eng = dma_lanes[(lane0 + do) % len(dma_lanes)]
                eng.dma_start(out=t[:, do, :], in_=v[:, do, :])
            return t

        xc_b0 = load_xT(xcT, 0, "xcT", lane0=0)
        w_sb = {"q": load_wT(wqT, "wq", lane0=2)}

        bqs_sb = consts.tile([P, DCH], F32)
        nc.sync.dma_start(out=bqs_sb, in_=bqs[:].rearrange("(o p) -> p o", p=P))

        xa_b0 = load_xT(xaT, 0, "xaT", lane0=1)
        w_sb["k"] = load_wT(wkT, "wk", lane0=3)
        xb_b0 = load_xT(xbT, 0, "xbT", lane0=0)
        w_sb["v"] = load_wT(wvT, "wv", lane0=2)
        w_sb["f"] = load_wT(wfT, "wf", lane0=1)

        bfc_sb = consts.tile([P, DIM], F32)
        nc.sync.dma_start(out=bfc_sb, in_=bfc[:].partition_broadcast(P))

        idn = consts.tile([P, P], BF16)
        make_identity(nc, idn)
        eps_sb = consts.tile([P, 1], F32)
        nc.vector.memset(eps_sb, EPS)
        eshift_sb = consts.tile([P, 1], F32)
        nc.vector.memset(eshift_sb, -ESHIFT)

        for b in range(BPC):
            if b == 0:
                x_sb = {"a": xa_b0, "b": xb_b0, "c": xc_b0}
            else:
                x_sb = {
                    "c": load_xT(xcT, b, "xcT", lane0=0),
                    "a": load_xT(xaT, b, "xaT", lane0=2),
                    "b": load_xT(xbT, b, "xbT", lane0=1),
                }

            # --- projections Q^T, K_A^T, K_B^T: [e, s] (e on partitions),
            # stored fp8 (values ~N(0,1), well inside e4m3 range); the softmax
            # 1/sqrt(D) scale is applied later inside the Exp activation ---
            def projT(tag, w_t, x_t, bias_ap=None):
                dst = qkv.tile([P, DCH, S], FP8, tag=tag)
                for ec in range(DCH):
                    for sh in range(S // 512):
                        ps = ps512.tile([P, 512], F32, tag="ps512")
                        for do in range(DCH):
                            nc.tensor.matmul(
                                ps,
                                lhsT=w_t[:, do, ec * P:(ec + 1) * P],
                                rhs=x_t[:, do, sh * 512:(sh + 1) * 512],
                                start=(do == 0),
                                stop=(do == DCH - 1),
                            )
                        o = dst[:, ec, sh * 512:(sh + 1) * 512]
                        if bias_ap is not None:
                            nc.scalar.activation(
                                out=o, in_=ps, func=AF.Identity,
                                bias=bias_ap[:, ec:ec + 1], scale=1.0,
                            )
                        else:
                            nc.scalar.copy(out=o, in_=ps)
                return dst

            qT_sb = projT("QT", w_sb["q"], x_sb["c"], bias_ap=bqs_sb)
            kT = {
                "a": projT("KAT", w_sb["k"], x_sb["a"]),
                "b": projT("KBT", w_sb["k"], x_sb["b"]),
            }

            # --- V_A, V_B natural layout [s, e] + two ones columns ---
            v_sb = {}
            for name in ("a", "b"):
                dst = qkv.tile([P, SCH, VW], FP8, tag=f"V{name.upper()}")
                nc.vector.memset(dst[:, :, DIM:VW], 1.0)
                for sc in range(SCH):
                    for off, w in ((0, 512), (512, 256)):
                        pool, pw = (ps512, 512) if w == 512 else (ps258, 258)
                        ps = pool.tile([P, pw], F32, tag=f"ps{pw}")
                        for do in range(DCH):
                            nc.tensor.matmul(
                                ps[:, :w],
                                lhsT=x_sb[name][:, do, sc * P:(sc + 1) * P],
                                rhs=w_sb["v"][:, do, off:off + w],
                                start=(do == 0),
                                stop=(do == DCH - 1),
                            )
                        nc.scalar.copy(out=dst[:, sc, off:off + w], in_=ps[:, :w])
                v_sb[name] = dst

            # --- attention + epilogue, per q-block of 512 ---
            for qb in range(S // 512):
                qsl = slice(qb * 512, (qb + 1) * 512)
                # scores^T and exp: e^T[k, q] = exp(K[k,:] @ Qs[q,:])
                e_sb = {}
                for name in ("a", "b"):
                    et = epool.tile([P, SCH, 512], FP8, tag=f"e{name.upper()}")
                    for kc in range(SCH):
                        ps = ps512.tile([P, 512], F32, tag="ps512")
                        for ep in range(DCH // 2):
                            esl = slice(2 * ep, 2 * ep + 2)
                            nc.tensor.matmul(
                                ps,
                                lhsT=kT[name][:, esl, kc * P:(kc + 1) * P],
                                rhs=qT_sb[:, esl, qsl],
                                start=(ep == 0),
                                stop=(ep == DCH // 2 - 1),
                                perf_mode=mybir.MatmulPerfMode.DoubleRow,
                            )
                        # exp((Q.K) / sqrt(D) - ESHIFT)
                        nc.scalar.activation(
                            out=et[:, kc, :], in_=ps, func=AF.Exp,
                            bias=eshift_sb, scale=SCALE,
                        )
                    e_sb[name] = et

                # pass 1: interp + layernorm -> z[qi]
                zs = []
                for qi in range(4):
                    qc = qb * 4 + qi
                    qs = slice(qi * P, (qi + 1) * P)

                    xc_t = opool.tile([P, DIM], F32, tag="xc")
                    nc.gpsimd.dma_start(out=xc_t, in_=xcr[b, qc * P:(qc + 1) * P, :])

                    # interp psums; h1 carries the ones columns -> row-sums
                    pa = {}
                    for name in ("a", "b"):
                        p0 = ps512.tile([P, 512], F32, tag="ps512")
                        p1 = ps258.tile([P, 258], F32, tag="ps258")
                        for kp in range(SCH // 2):
                            ksl = slice(2 * kp, 2 * kp + 2)
                            nc.tensor.matmul(
                                p0,
                                lhsT=e_sb[name][:, ksl, qs],
                                rhs=v_sb[name][:, ksl, 0:512],
                                start=(kp == 0),
                                stop=(kp == SCH // 2 - 1),
                                perf_mode=mybir.MatmulPerfMode.DoubleRow,
                            )
                        for kp in range(SCH // 2):
                            ksl = slice(2 * kp, 2 * kp + 2)
                            nc.tensor.matmul(
                                p1,
                                lhsT=e_sb[name][:, ksl, qs],
                                rhs=v_sb[name][:, ksl, 512:VW],
                                start=(kp == 0),
                                stop=(kp == SCH // 2 - 1),
                                perf_mode=mybir.MatmulPerfMode.DoubleRow,
                            )
                        pa[name] = (p0, p1)

                    rcp = {}
                    for name in ("a", "b"):
                        r = spool.tile([P, 1], F32, tag=f"r{name}")
                        nc.vector.reciprocal(r, pa[name][1][:, 256:257])
                        rcp[name] = r

                    # t1 = psA*rA + xc ; t1 += psB*rB   (residual fused)
                    t1 = spool.tile([P, DIM], F32, tag="t1")
                    for (off, w, pi) in ((0, 512, 0), (512, 256, 1)):
                        nc.vector.scalar_tensor_tensor(
                            out=t1[:, off:off + w],
                            in0=pa["a"][pi][:, 0:w] if pi == 0 else pa["a"][1][:, 0:256],
                            scalar=rcp["a"], in1=xc_t[:, off:off + w],
                            op0=ALU.mult, op1=ALU.add,
                        )
                        nc.vector.scalar_tensor_tensor(
                            out=t1[:, off:off + w],
                            in0=pa["b"][pi][:, 0:w] if pi == 0 else pa["b"][1][:, 0:256],
                            scalar=rcp["b"], in1=t1[:, off:off + w],
                            op0=ALU.mult, op1=ALU.add,
                        )

                    # layernorm
                    stats = spool.tile([P, 3, 6], F32, tag="st")
                    for g in range(3):
                        nc.vector.bn_stats(
                            out=stats[:, g, :], in_=t1[:, g * 256:(g + 1) * 256]
                        )
                    mv = spool.tile([P, 2], F32, tag="mv")
                    nc.vector.bn_aggr(out=mv, in_=stats)
                    std = spool.tile([P, 1], F32, tag="std")
                    nc.scalar.activation(
                        out=std, in_=mv[:, 1:2], func=AF.Sqrt, bias=eps_sb
                    )
                    rstd = spool.tile([P, 1], F32, tag="rstd")
                    nc.vector.reciprocal(rstd, std)
                    z = zpool.tile([P, DIM], BF16, tag="z")
                    nc.vector.tensor_scalar(
                        out=z, in0=t1,
                        scalar1=mv[:, 0:1], scalar2=rstd,
                        op0=ALU.subtract, op1=ALU.mult,
                    )
                    zs.append(z)

                # pass 2: transpose h + fc + store
                for qi in range(4):
                    qc = qb * 4 + qi
                    z = zs[qi]

                    hT = opool.tile([P, DCH, P], BF16, tag="hT")
                    for ep in range(DCH // 2):
                        pst = pstr.tile([P, 2, P], BF16, tag="pstr")
                        for j in range(2):
                            eo = ep * 2 + j
                            nc.tensor.transpose(
                                pst[:, j], z[:, eo * P:(eo + 1) * P], idn
                            )
                        nc.scalar.copy(out=hT[:, ep * 2:(ep + 1) * 2, :], in_=pst)

                    o_t = opool.tile([P, DIM], F32, tag="o")
                    for off, w in ((0, 512), (512, 256)):
                        pool, pw = (ps512, 512) if w == 512 else (ps258, 258)
                        ps = pool.tile([P, pw], F32, tag=f"ps{pw}")
                        for eo in range(DCH):
                            nc.tensor.matmul(
                                ps[:, :w],
                                lhsT=hT[:, eo, :],
                                rhs=w_sb["f"][:, eo, off:off + w],
                                start=(eo == 0),
                                stop=(eo == DCH - 1),
                            )
                        nc.vector.tensor_add(
                            o_t[:, off:off + w], ps[:, :w], bfc_sb[:, off:off + w]
                        )
                    nc.sync.dma_start(out=out[b, qc * P:(qc + 1) * P, :], in_=o_t)

    nc.compile()
    return nc


_CACHED_NC = None
_LAST_IN_MAPS = None


def kernel(**inputs) -> np.ndarray:
    global _CACHED_NC, _LAST_IN_MAPS
    bf16 = ml_dtypes.bfloat16
    f32 = np.float32

    xA = np.asarray(inputs["x_A"], dtype=f32)
    xB = np.asarray(inputs["x_B"], dtype=f32)
    xC = np.asarray(inputs["x_C"], dtype=f32)

    xaT = np.ascontiguousarray(xA.transpose(0, 2, 1)).astype(bf16)
    xbT = np.ascontiguousarray(xB.transpose(0, 2, 1)).astype(bf16)
    xcT = np.ascontiguousarray(xC.transpose(0, 2, 1)).astype(bf16)
    xcr = (xC + 2.0 * np.asarray(inputs["b_v"], dtype=f32)).astype(f32)

    wqT = np.ascontiguousarray(np.asarray(inputs["W_q"], dtype=f32).T).astype(bf16)
    wkT = np.ascontiguousarray(np.asarray(inputs["W_k"], dtype=f32).T).astype(bf16)
    wvT = np.ascontiguousarray(np.asarray(inputs["W_v"], dtype=f32).T).astype(bf16)

    # fold LayerNorm's gamma/beta into the fc layer (exact):
    #   h = z*gamma + beta;  out = h @ W_fc.T + b_fc
    #     = z @ (W_fc * gamma).T + (b_fc + W_fc @ beta)
    gam = np.asarray(inputs["gamma"], dtype=f32)
    bet = np.asarray(inputs["beta"], dtype=f32)
    W_fc = np.asarray(inputs["W_fc"], dtype=f32)
    wfT = np.ascontiguousarray(W_fc.T * gam[:, None]).astype(bf16)
    bfc = (np.asarray(inputs["b_fc"], dtype=f32) + W_fc @ bet).astype(f32)

    # unscaled: the softmax 1/sqrt(D) factor is applied inside the Exp
    # activation on device (keeps fp8-stored Q in a healthy value range)
    bqs = np.asarray(inputs["b_q"], dtype=f32)

    if _CACHED_NC is None:
        _CACHED_NC = build_bass()
    nc = _CACHED_NC

    in_maps = []
    for c in range(NCORES):
        sl = slice(c * BPC, (c + 1) * BPC)
        in_maps.append({
            "xaT": np.ascontiguousarray(xaT[sl]),
            "xbT": np.ascontiguousarray(xbT[sl]),
            "xcT": np.ascontiguousarray(xcT[sl]),
            "xcr": np.ascontiguousarray(xcr[sl]),
            "wqT": wqT, "wkT": wkT, "wvT": wvT, "wfT": wfT,
            "bqs": bqs, "bfc": bfc,
        })

    _LAST_IN_MAPS = in_maps
    res = run_bass_kernel_spmd(nc, in_maps, core_ids=list(range(NCORES)))
    outs = [np.asarray(res.results[i]["out"], dtype=f32) for i in range(NCORES)]
    return np.concatenate(outs, axis=0)


if __name__ == "__main__":
    rng = np.random.default_rng(0)
    fake = {
        "x_A": rng.standard_normal((B, S, DIM), dtype=np.float32),
        "x_B": rng.standard_normal((B, S, DIM), dtype=np.float32),
        "x_C": rng.standard_normal((B, S, DIM), dtype=np.float32),
        "W_q": rng.standard_normal((DIM, DIM), dtype=np.float32) / 27.7,
        "b_q": rng.standard_normal(DIM).astype(np.float32) / 27.7,
        "W_k": rng.standard_normal((DIM, DIM), dtype=np.float32) / 27.7,
        "b_k": rng.standard_normal(DIM).astype(np.float32) / 27.7,
        "W_v": rng.standard_normal((DIM, DIM), dtype=np.float32) / 27.7,
        "b_v": rng.standard_normal(DIM).astype(np.float32) / 27.7,
        "gamma": np.ones(DIM, np.float32),
        "beta": np.zeros(DIM, np.float32),
        "W_fc": rng.standard_normal((DIM, DIM), dtype=np.float32) / 27.7,
        "b_fc": rng.standard_normal(DIM).astype(np.float32) / 27.7,
    }
    o = kernel(**fake)
    print(o.shape, o.dtype)
